# revision 44
# baseline (speedup 1.0000x reference)
"""FRQI encoding kernel for Trainium2 (8 NeuronCores, data-parallel).

Closed form of the reference: for each sample b with 4 pixels x[b, 0:4],
  out[b] = [0.0, 0.0, mean_i cos(x[b, i] * pi / 255)]
The two address-qubit columns are input-independent and exactly zero
(mean over 4 pixel indices of (-1)^bit is 0 for both address bits), so
the device only computes and ships the color column; the constant zero
columns are materialized host-side during unsharding. The color column
is stored as fp16 (rel-err contribution ~2e-4, two orders under the
2e-2 gate), cutting per-core HBM traffic from 14 MiB (8 in + 6 out,
f32 interleaved) to 9 MiB (8 in + 1 out).

Device kernel (per core, 524288 samples = 2097152 input floats):
  - tiles of (128 partitions x F floats), contiguous DMA in; ALL loads
    on the ACT-engine DGE ring, dispatched back-to-back before the
    first activation (loads carry no sem waits; a single queue sustains
    ~400 GB/s where two queues measured ~300; any compute interleaved
    into the dispatch burst stalls the stream and costs ~5 us)
  - F schedule [512, 4096x3, 2048, 1024, 512]: 16 KiB partition lines
    for the bulk (DMA efficiency), small first tile so the Sin-table
    load + first act + DVE pipeline start ~8 us early, decreasing tail
    so the post-last-load drain is one small tile's chain
  - ScalarE activation Sin(pi/2 - x*pi/255) == +cos(2*theta), in-place
    (the HW Sin spline is only accurate on ~[-pi, pi]; the +pi/2 bias
    keeps arguments in (-pi/2, pi/2])
  - VectorE grouped sum of 4 as two pairwise stride-2 tensor_adds
    (tensor_tensor cost tracks OUTPUT size: 0.75*F cycles vs reduce's
    F), the second writing fp16 directly; host applies the mean's *0.25
    during the f32 upcast (exact to within fp16 rounding)
  - stores on the Sync DGE ring right after each tile's add2, fully
    overlapped with the remaining loads
  - Bacc(enable_partition_id=False, monotonic_sem_count=0) plus a
    barrier-free TileContext exit (drain + sem clears all on Sync, in
    program order) trim ~4 us of fixed init/teardown
"""

import math
import sys

for _p in ("/opt/trn_rl_repo",):
    if _p not in sys.path:
        sys.path.append(_p)

import numpy as np

# If the environment forces tracing (BASS_TRACE=1), run_bass_kernel_spmd
# imports antenv.axon_hooks, which this image lacks — stub it (only when
# absent) so the trace path degrades to "hook isn't registered" instead
# of crashing the kernel.
try:
    import antenv.axon_hooks  # noqa: F401
except ImportError:
    import types as _types

    _m = _types.ModuleType("antenv.axon_hooks")
    _m.get_axon_ntff_profile_hook = lambda: None
    _m.set_axon_ntff_profile_hook = lambda h: None
    sys.modules["antenv.axon_hooks"] = _m

import concourse.bass as bass
import concourse.mybir as mybir
from concourse import bacc
from concourse.bass_utils import run_bass_kernel_spmd
from concourse.tile import TileContext
from concourse.vector_clock import ScopedClock

N_CORES = 8
B = 4_194_304
N_PIX = 4
N_PER_CORE = B // N_CORES          # 524288 samples
P = 128                            # SBUF partitions
L = N_PER_CORE * N_PIX             # 2097152 input floats per core

# Per-tile free-dim sizes (floats per partition). DMA efficiency is
# driven by the per-partition line length (F*4 bytes): 16 KiB lines
# sustain ~400 GB/s aggregate while 4 KiB lines drop to ~300 (measured)
# — so the bulk must be 4096-wide tiles. The small FIRST tile starts
# the ACT/DVE pipeline ~8 us earlier (its act is what gates the Sin
# table load and every downstream DVE op); the decreasing TAIL keeps
# the post-last-load drain (last act + adds + store) short.
F_SCHED = [512, 4096, 4096, 4096, 2048, 1024, 512]
assert sum(F_SCHED) * P == L

# COMPUTE sub-tiling, decoupled from the DMA tiling: every sub-slice
# of a landed tile waits on the same load semaphore, so act/adds/store
# pipeline across sub-slices WITHIN a DMA tile (act of sub k+1 overlaps
# adds of sub k on DVE). This cuts the post-last-load drain from a full
# 4096-wide act+add chain to act_total + the LAST SUB's adds, without
# giving up 16 KiB DMA lines the way a finer DMA tiling would. Early
# big tiles stay unsplit: their chains hide under the load stream, and
# fewer instructions = less fixed ACT/DVE per-op overhead.
SUB_SCHED = [
    [512],
    [4096],
    [4096],
    [2048, 1024, 1024],
    [2048],
    [1024],
    [512],
]
assert all(sum(s) == F for s, F in zip(SUB_SCHED, F_SCHED))

# cos(z) = sin(pi/2 - z) for z = x*pi/255 = 2*theta: with scale=-pi/255
# and bias=+pi/2 the activation argument stays in (-pi/2, pi/2], the
# accurate domain of the HW Sin spline (it degrades badly beyond ~pi),
# and no sign fix-up is needed downstream.
_SCALE = -math.pi / 255.0
_BIAS = math.pi / 2.0


def _make_bacc() -> bacc.Bacc:
    """Construct Bacc without its init-time const-AP memsets and
    all-engine barrier. Nothing reads the four built-in const APs here
    (the activation bias is an explicitly-memset SBUF tensor, never a
    float — a float bias would route through the const APs and read
    uninitialized SBUF), and without the barrier each engine reaches its
    first kernel instruction as soon as its own runtime prolog finishes.
    The patched methods are restored before any kernel instruction is
    traced."""
    sh = bass.BassSharedVectorInterface
    saved_memset = sh.memset
    saved_barrier = bass.Bass.all_engine_barrier
    sh.memset = lambda self, ap, constant: None
    bass.Bass.all_engine_barrier = lambda self, *a, **k: None
    # (Tried: shrinking the kernel's declared semaphore pool to ~40 —
    # the NEFF exit still resets the full [2,256) space per engine, so
    # the ~6 us reset storm is hardcoded downstream. Reverted.)
    try:
        # No partition-id reads and no monotonic sems in this kernel:
        # disabling them drops the per-engine init TENSOR_LOADs (~1.2 us
        # of register setup before the first DMA dispatch).
        return bacc.Bacc(enable_partition_id=False, monotonic_sem_count=0)
    finally:
        sh.memset = saved_memset
        bass.Bass.all_engine_barrier = saved_barrier


def _fast_drain_and_barrier(self, tick_clock, wait_clock):
    """Barrier-free replacement for TileContext._drain_and_barrier.

    The stock exit emits drain + all-engine barrier + gpsimd sem clears
    + another all-engine barrier (~2-3 us of engine-skew waits at the
    very end of the kernel). The barriers only exist to order the
    gpsimd-issued clears against the other engines; issuing the drain
    AND the clears on the Sync engine instead makes program order carry
    that dependency: the drain waits on every outstanding DMA/compute
    semaphore, and the clears follow it in Sync's own stream. The
    NEFF-level postamble (which resets the whole semaphore space
    per-engine) still runs after, so cross-run state is unchanged."""
    nc = self.nc
    drain_inst = nc.sync.drain()
    wait_clock.add_sem_waits(
        drain_inst.ins, ScopedClock({None: tick_clock.global_clock})
    )
    popped = nc._tile_sem_poison_stack.pop()
    assert popped is self._sem_poison
    sems = list(self.sems.allocated().values())
    sem_nums = [s.num if hasattr(s, "num") else s for s in sems]
    for sem_range in bass.compact_to_ranges(sem_nums):
        assert nc._state.free_isdisjoint(sem_range)
        nc.sync.drain(semaphore_range=sem_range)  # dma_reset equivalent
        nc.sync.sem_clear(sem_range)
    nc._state.prepend_free_semaphores(sem_nums)
    for poison_set in nc._tile_sem_poison_stack:
        poison_set.update(sem_nums)


def _build_nc() -> bass.Bass:
    # Bacc (not raw Bass): its compile() pass generate_event_semaphores
    # splits multi-sem waits to satisfy the 1-wait-per-instruction HW limit.
    nc = _make_bacc()
    f32 = mybir.dt.float32
    f16 = mybir.dt.float16
    bf16 = mybir.dt.bfloat16
    x = nc.dram_tensor("x", [L], f32, kind="ExternalInput")
    y = nc.dram_tensor("y", [N_PER_CORE], f16, kind="ExternalOutput")

    bias_t = nc.alloc_sbuf_tensor("bias_pi2", [P, 1], f32)
    bias_ap = bias_t.ap()

    with TileContext(nc) as tc:
        # One slot per uniquely-tagged tile: no slot reuse, so no in-DMA
        # ever carries a WAR wait and the ACT sequencer can dispatch
        # every input DMA up front; slots are sized per tile (a shared
        # tag would size every slot to the largest tile).
        with tc.tile_pool(name="io", bufs=1) as pool:
            nc.gpsimd.memset(bias_ap, _BIAS)
            # All load dispatches FIRST in the ACT stream, before any
            # activation, and all on ONE queue: splitting loads across
            # two DGE queues was measured at ~300 GB/s aggregate vs
            # ~400 for a single queue (two interleaved address streams
            # lose HBM sequentiality across the shared engine pool).
            # Loads carry no sem waits, so the 7 dispatches retire
            # back-to-back before the sequencer blocks on the first
            # activation.
            # All 7 load dispatches FIRST, back-to-back: emitting any
            # compute between them (e.g. act0 after two dispatches, to
            # start DVE earlier) was measured 5+ us SLOWER — the ACT
            # sequencer blocks on act0's load wait, the load queue goes
            # briefly idle, and the stream loses its ramp/arbitration.
            in_tiles = []
            in_off = 0
            for t, F in enumerate(F_SCHED):
                x_t = x[in_off:in_off + P * F].rearrange("(p f) -> p f", p=P)
                it = pool.tile([P, F], f32, tag=f"in{t}")
                nc.scalar.dma_start(out=it[:], in_=x_t)
                in_tiles.append(it)
                in_off += P * F
            out_off = 0
            for t, F in enumerate(F_SCHED):
                it = in_tiles[t]
                C = F // N_PIX
                # Per-partition output segments are contiguous per DMA
                # TILE (stride C), so a sub-slice's outputs form a
                # COLUMN slice of the tile's (P, C) view — not a packed
                # contiguous range.
                y_tile = y[out_off:out_off + P * C].rearrange(
                    "(p c) -> p c", p=P
                )
                # NOTE: routing the Sin output through a separate bf16
                # tile (for 2x 16-bit DVE adds) was measured to produce
                # INTERMITTENT wrong results (rel err 0.1 on 1 of 3
                # runs — a missed dependency race) with no speed gain;
                # keep the in-place f32 activation.
                st = pool.tile([P, C], f16, tag=f"sum{t}")
                g = 0
                for s, S in enumerate(SUB_SCHED[t]):
                    SC = S // N_PIX
                    sub = it[:, g:g + S]
                    nc.scalar.activation(
                        sub, sub, mybir.ActivationFunctionType.Sin,
                        bias=bias_ap, scale=_SCALE,
                    )
                    # Grouped sum of 4 as two pairwise adds; the second
                    # writes fp16 directly (DVE converts on write) into
                    # this sub's column slice of the tile's output
                    # buffer; the host applies the mean's *0.25 during
                    # the f32 upcast. Unique tags: no WAR waits.
                    pt = pool.tile([P, S // 2], f32, tag=f"pair{t}_{s}")
                    nc.vector.tensor_add(
                        pt[:], it[:, g:g + S:2], it[:, g + 1:g + S:2]
                    )
                    nc.vector.tensor_add(
                        st[:, g // N_PIX:g // N_PIX + SC],
                        pt[:, 0:S // 2:2], pt[:, 1:S // 2:2]
                    )
                    g += S
                # ONE store per DMA tile (not per sub), after the last
                # sub's add2: fewer serialized dispatches on the Sync
                # ring; the subs' adds complete nearly together anyway.
                nc.sync.dma_start(out=y_tile, in_=st[:])
                out_off += P * C
    nc.finalize()
    return nc


def _build_nc_patched() -> bass.Bass:
    saved = TileContext._drain_and_barrier
    TileContext._drain_and_barrier = _fast_drain_and_barrier
    try:
        return _build_nc()
    finally:
        TileContext._drain_and_barrier = saved


_NC_CACHE = None


def _get_nc() -> bass.Bass:
    global _NC_CACHE
    if _NC_CACHE is None:
        _NC_CACHE = _build_nc_patched()
    return _NC_CACHE


def _run(x: np.ndarray, **spmd_kwargs):
    """x: (B, 4) float32. Returns (full_output, BassKernelResults)."""
    shards = x.reshape(N_CORES, L)
    in_maps = [{"x": shards[i]} for i in range(N_CORES)]
    res = run_bass_kernel_spmd(_get_nc(), in_maps, list(range(N_CORES)), **spmd_kwargs)
    out = np.zeros((B, 3), dtype=np.float32)
    for i, r in enumerate(res.results):
        # Device ships the per-sample sum of the 4 cosines (fp16); the
        # mean's *0.25 is applied here during the f32 upcast.
        out[i * N_PER_CORE:(i + 1) * N_PER_CORE, 2] = np.asarray(
            r["y"], dtype=np.float32
        ).reshape(N_PER_CORE) * np.float32(0.25)
    return out, res


def kernel(**inputs: np.ndarray) -> np.ndarray:
    x = np.ascontiguousarray(
        np.asarray(inputs["inputs"], dtype=np.float32)
    ).reshape(B, N_PIX)
    out, _ = _run(x)
    return out


# revision 45
# speedup vs baseline: 1.1018x; 1.1018x over previous
"""FRQI encoding kernel for Trainium2 (8 NeuronCores, data-parallel).

Closed form of the reference: for each sample b with 4 pixels x[b, 0:4],
  out[b] = [0.0, 0.0, mean_i cos(x[b, i] * pi / 255)]
The two address-qubit columns are input-independent and exactly zero
(mean over 4 pixel indices of (-1)^bit is 0 for both address bits), so
the device only computes and ships the color column; the constant zero
columns are materialized host-side during unsharding. The color column
is stored as fp16 (rel-err contribution ~2e-4, two orders under the
2e-2 gate), cutting per-core HBM traffic from 14 MiB (8 in + 6 out,
f32 interleaved) to 9 MiB (8 in + 1 out).

Device kernel (per core, 524288 samples = 2097152 input floats):
  - tiles of (128 partitions x F floats), contiguous DMA in; ALL loads
    on the ACT-engine DGE ring, dispatched back-to-back before the
    first activation (loads carry no sem waits; a single queue sustains
    ~400 GB/s where two queues measured ~300; any compute interleaved
    into the dispatch burst stalls the stream and costs ~5 us)
  - F schedule [512, 4096x3, 2048, 1024, 512]: 16 KiB partition lines
    for the bulk (DMA efficiency), small first tile so the Sin-table
    load + first act + DVE pipeline start ~8 us early, decreasing tail
    so the post-last-load drain is one small tile's chain
  - ScalarE activation Sin(pi/2 - x*pi/255) == +cos(2*theta), in-place
    (the HW Sin spline is only accurate on ~[-pi, pi]; the +pi/2 bias
    keeps arguments in (-pi/2, pi/2])
  - VectorE grouped sum of 4 as two pairwise stride-2 tensor_adds
    (tensor_tensor cost tracks OUTPUT size: 0.75*F cycles vs reduce's
    F), the second writing fp16 directly; host applies the mean's *0.25
    during the f32 upcast (exact to within fp16 rounding)
  - stores on the Sync DGE ring right after each tile's add2, fully
    overlapped with the remaining loads
  - Bacc(enable_partition_id=False, monotonic_sem_count=0) plus a
    barrier-free TileContext exit (drain + sem clears all on Sync, in
    program order) trim ~4 us of fixed init/teardown
"""

import math
import sys

for _p in ("/opt/trn_rl_repo",):
    if _p not in sys.path:
        sys.path.append(_p)

import numpy as np

# If the environment forces tracing (BASS_TRACE=1), run_bass_kernel_spmd
# imports antenv.axon_hooks, which this image lacks — stub it (only when
# absent) so the trace path degrades to "hook isn't registered" instead
# of crashing the kernel.
try:
    import antenv.axon_hooks  # noqa: F401
except ImportError:
    import types as _types

    _m = _types.ModuleType("antenv.axon_hooks")
    _m.get_axon_ntff_profile_hook = lambda: None
    _m.set_axon_ntff_profile_hook = lambda h: None
    sys.modules["antenv.axon_hooks"] = _m

import concourse.bass as bass
import concourse.mybir as mybir
from concourse import bacc
from concourse.bass_utils import run_bass_kernel_spmd
from concourse.tile import TileContext
from concourse.vector_clock import ScopedClock

N_CORES = 8
B = 4_194_304
N_PIX = 4
N_PER_CORE = B // N_CORES          # 524288 samples
P = 128                            # SBUF partitions
L = N_PER_CORE * N_PIX             # 2097152 input floats per core

# Per-tile free-dim sizes (floats per partition). DMA efficiency is
# driven by the per-partition line length (F*4 bytes): 16 KiB lines
# sustain ~400 GB/s aggregate while 4 KiB lines drop to ~300 (measured)
# — so the bulk must be 4096-wide tiles. The small FIRST tile starts
# the ACT/DVE pipeline ~8 us earlier (its act is what gates the Sin
# table load and every downstream DVE op); the decreasing TAIL keeps
# the post-last-load drain (last act + adds + store) short.
F_SCHED = [512, 4096, 4096, 4096, 2048, 1024, 512]
assert sum(F_SCHED) * P == L

# COMPUTE sub-tiling, decoupled from the DMA tiling: every sub-slice
# of a landed tile waits on the same load semaphore, so act/adds/store
# pipeline across sub-slices WITHIN a DMA tile (act of sub k+1 overlaps
# adds of sub k on DVE). This cuts the post-last-load drain from a full
# 4096-wide act+add chain to act_total + the LAST SUB's adds, without
# giving up 16 KiB DMA lines the way a finer DMA tiling would. Early
# big tiles stay unsplit: their chains hide under the load stream, and
# fewer instructions = less fixed ACT/DVE per-op overhead.
SUB_SCHED = [
    [512],
    [4096],
    [4096],
    [2048, 1024, 1024],
    [2048],
    [1024],
    [512],
]
assert all(sum(s) == F for s, F in zip(SUB_SCHED, F_SCHED))

# cos(z) = sin(pi/2 - z) for z = x*pi/255 = 2*theta: with scale=-pi/255
# and bias=+pi/2 the activation argument stays in (-pi/2, pi/2], the
# accurate domain of the HW Sin spline (it degrades badly beyond ~pi),
# and no sign fix-up is needed downstream.
_SCALE = -math.pi / 255.0
_BIAS = math.pi / 2.0


def _make_bacc() -> bacc.Bacc:
    """Construct Bacc without its init-time const-AP memsets and
    all-engine barrier. Nothing reads the four built-in const APs here
    (the activation bias is an explicitly-memset SBUF tensor, never a
    float — a float bias would route through the const APs and read
    uninitialized SBUF), and without the barrier each engine reaches its
    first kernel instruction as soon as its own runtime prolog finishes.
    The patched methods are restored before any kernel instruction is
    traced."""
    sh = bass.BassSharedVectorInterface
    saved_memset = sh.memset
    saved_barrier = bass.Bass.all_engine_barrier
    sh.memset = lambda self, ap, constant: None
    bass.Bass.all_engine_barrier = lambda self, *a, **k: None
    # (Tried: shrinking the kernel's declared semaphore pool to ~40 —
    # the NEFF exit still resets the full [2,256) space per engine, so
    # the ~6 us reset storm is hardcoded downstream. Reverted.)
    try:
        # No partition-id reads and no monotonic sems in this kernel:
        # disabling them drops the per-engine init TENSOR_LOADs (~1.2 us
        # of register setup before the first DMA dispatch).
        return bacc.Bacc(enable_partition_id=False, monotonic_sem_count=0)
    finally:
        sh.memset = saved_memset
        bass.Bass.all_engine_barrier = saved_barrier


def _fast_drain_and_barrier(self, tick_clock, wait_clock):
    """Barrier-free replacement for TileContext._drain_and_barrier.

    The stock exit emits drain + all-engine barrier + gpsimd sem clears
    + another all-engine barrier (~2-3 us of engine-skew waits at the
    very end of the kernel). The barriers only exist to order the
    gpsimd-issued clears against the other engines; issuing the drain
    AND the clears on the Sync engine instead makes program order carry
    that dependency: the drain waits on every outstanding DMA/compute
    semaphore, and the clears follow it in Sync's own stream. The
    NEFF-level postamble (which resets the whole semaphore space
    per-engine) still runs after, so cross-run state is unchanged."""
    nc = self.nc
    drain_inst = nc.sync.drain()
    wait_clock.add_sem_waits(
        drain_inst.ins, ScopedClock({None: tick_clock.global_clock})
    )
    popped = nc._tile_sem_poison_stack.pop()
    assert popped is self._sem_poison
    sems = list(self.sems.allocated().values())
    sem_nums = [s.num if hasattr(s, "num") else s for s in sems]
    for sem_range in bass.compact_to_ranges(sem_nums):
        assert nc._state.free_isdisjoint(sem_range)
        nc.sync.drain(semaphore_range=sem_range)  # dma_reset equivalent
        nc.sync.sem_clear(sem_range)
    nc._state.prepend_free_semaphores(sem_nums)
    for poison_set in nc._tile_sem_poison_stack:
        poison_set.update(sem_nums)


def _build_nc() -> bass.Bass:
    # Bacc (not raw Bass): its compile() pass generate_event_semaphores
    # splits multi-sem waits to satisfy the 1-wait-per-instruction HW limit.
    nc = _make_bacc()
    f32 = mybir.dt.float32
    f16 = mybir.dt.float16
    bf16 = mybir.dt.bfloat16
    x = nc.dram_tensor("x", [L], f32, kind="ExternalInput")
    y = nc.dram_tensor("y", [N_PER_CORE], f16, kind="ExternalOutput")

    bias_t = nc.alloc_sbuf_tensor("bias_pi2", [P, 1], f32)
    bias_ap = bias_t.ap()

    with TileContext(nc) as tc:
        # One slot per uniquely-tagged tile: no slot reuse, so no in-DMA
        # ever carries a WAR wait and the ACT sequencer can dispatch
        # every input DMA up front; slots are sized per tile (a shared
        # tag would size every slot to the largest tile).
        with tc.tile_pool(name="io", bufs=1) as pool:
            nc.gpsimd.memset(bias_ap, _BIAS)
            # All load dispatches FIRST in the ACT stream, before any
            # activation, and all on ONE queue: splitting loads across
            # two DGE queues was measured at ~300 GB/s aggregate vs
            # ~400 for a single queue (two interleaved address streams
            # lose HBM sequentiality across the shared engine pool).
            # Loads carry no sem waits, so the 7 dispatches retire
            # back-to-back before the sequencer blocks on the first
            # activation.
            # All 7 load dispatches FIRST, back-to-back: emitting any
            # compute between them (e.g. act0 after two dispatches, to
            # start DVE earlier) was measured 5+ us SLOWER — the ACT
            # sequencer blocks on act0's load wait, the load queue goes
            # briefly idle, and the stream loses its ramp/arbitration.
            in_tiles = []
            in_off = 0
            for t, F in enumerate(F_SCHED):
                x_t = x[in_off:in_off + P * F].rearrange("(p f) -> p f", p=P)
                it = pool.tile([P, F], f32, tag=f"in{t}")
                nc.scalar.dma_start(out=it[:], in_=x_t)
                in_tiles.append(it)
                in_off += P * F
            out_off = 0
            for t, F in enumerate(F_SCHED):
                it = in_tiles[t]
                C = F // N_PIX
                # Per-partition output segments are contiguous per DMA
                # TILE (stride C), so a sub-slice's outputs form a
                # COLUMN slice of the tile's (P, C) view — not a packed
                # contiguous range.
                y_tile = y[out_off:out_off + P * C].rearrange(
                    "(p c) -> p c", p=P
                )
                # NOTE: routing the Sin output through a separate bf16
                # tile (for 2x 16-bit DVE adds) was measured to produce
                # INTERMITTENT wrong results (rel err 0.1 on 1 of 3
                # runs — a missed dependency race) with no speed gain;
                # keep the in-place f32 activation.
                st = pool.tile([P, C], f16, tag=f"sum{t}")
                g = 0
                for s, S in enumerate(SUB_SCHED[t]):
                    SC = S // N_PIX
                    sub = it[:, g:g + S]
                    nc.scalar.activation(
                        sub, sub, mybir.ActivationFunctionType.Sin,
                        bias=bias_ap, scale=_SCALE,
                    )
                    # Grouped sum of 4 as two pairwise adds; the second
                    # writes fp16 directly (DVE converts on write) into
                    # this sub's column slice of the tile's output
                    # buffer; the host applies the mean's *0.25 during
                    # the f32 upcast. Unique tags: no WAR waits.
                    pt = pool.tile([P, S // 2], f32, tag=f"pair{t}_{s}")
                    # add1 for the two EARLY unsplit 4096s runs on the
                    # otherwise-idle GpSimd engine: DVE is the saturated
                    # engine in the post-load tail (~2.5 us of queued
                    # adds when the last act retires), and these two
                    # tiles have ~6 us of deadline slack, so even
                    # GpSimd's slow software loop finishes well before
                    # DVE needs the pair sums. Cuts DVE busy by ~4 us.
                    a1_eng = nc.gpsimd if t in (1, 2) else nc.vector
                    a1_eng.tensor_add(
                        pt[:], it[:, g:g + S:2], it[:, g + 1:g + S:2]
                    )
                    nc.vector.tensor_add(
                        st[:, g // N_PIX:g // N_PIX + SC],
                        pt[:, 0:S // 2:2], pt[:, 1:S // 2:2]
                    )
                    g += S
                # ONE store per DMA tile (not per sub), after the last
                # sub's add2: fewer serialized dispatches on the Sync
                # ring; the subs' adds complete nearly together anyway.
                nc.sync.dma_start(out=y_tile, in_=st[:])
                out_off += P * C
    nc.finalize()
    return nc


def _build_nc_patched() -> bass.Bass:
    saved = TileContext._drain_and_barrier
    TileContext._drain_and_barrier = _fast_drain_and_barrier
    try:
        return _build_nc()
    finally:
        TileContext._drain_and_barrier = saved


_NC_CACHE = None


def _get_nc() -> bass.Bass:
    global _NC_CACHE
    if _NC_CACHE is None:
        _NC_CACHE = _build_nc_patched()
    return _NC_CACHE


def _run(x: np.ndarray, **spmd_kwargs):
    """x: (B, 4) float32. Returns (full_output, BassKernelResults)."""
    shards = x.reshape(N_CORES, L)
    in_maps = [{"x": shards[i]} for i in range(N_CORES)]
    res = run_bass_kernel_spmd(_get_nc(), in_maps, list(range(N_CORES)), **spmd_kwargs)
    out = np.zeros((B, 3), dtype=np.float32)
    for i, r in enumerate(res.results):
        # Device ships the per-sample sum of the 4 cosines (fp16); the
        # mean's *0.25 is applied here during the f32 upcast.
        out[i * N_PER_CORE:(i + 1) * N_PER_CORE, 2] = np.asarray(
            r["y"], dtype=np.float32
        ).reshape(N_PER_CORE) * np.float32(0.25)
    return out, res


def kernel(**inputs: np.ndarray) -> np.ndarray:
    x = np.ascontiguousarray(
        np.asarray(inputs["inputs"], dtype=np.float32)
    ).reshape(B, N_PIX)
    out, _ = _run(x)
    return out


# revision 46
# speedup vs baseline: 1.1097x; 1.0072x over previous
"""FRQI encoding kernel for Trainium2 (8 NeuronCores, data-parallel).

Closed form of the reference: for each sample b with 4 pixels x[b, 0:4],
  out[b] = [0.0, 0.0, mean_i cos(x[b, i] * pi / 255)]
The two address-qubit columns are input-independent and exactly zero
(mean over 4 pixel indices of (-1)^bit is 0 for both address bits), so
the device only computes and ships the color column; the constant zero
columns are materialized host-side during unsharding. The color column
is stored as fp16 (rel-err contribution ~2e-4, two orders under the
2e-2 gate), cutting per-core HBM traffic from 14 MiB (8 in + 6 out,
f32 interleaved) to 9 MiB (8 in + 1 out).

Device kernel (per core, 524288 samples = 2097152 input floats):
  - tiles of (128 partitions x F floats), contiguous DMA in; ALL loads
    on the ACT-engine DGE ring, dispatched back-to-back before the
    first activation (loads carry no sem waits; a single queue sustains
    ~400 GB/s where two queues measured ~300; any compute interleaved
    into the dispatch burst stalls the stream and costs ~5 us)
  - F schedule [512, 4096x3, 2048, 1024, 512]: 16 KiB partition lines
    for the bulk (DMA efficiency), small first tile so the Sin-table
    load + first act + DVE pipeline start ~8 us early, decreasing tail
    so the post-last-load drain is one small tile's chain
  - ScalarE activation Sin(pi/2 - x*pi/255) == +cos(2*theta), in-place
    (the HW Sin spline is only accurate on ~[-pi, pi]; the +pi/2 bias
    keeps arguments in (-pi/2, pi/2])
  - VectorE grouped sum of 4 as two pairwise stride-2 tensor_adds
    (tensor_tensor cost tracks OUTPUT size: 0.75*F cycles vs reduce's
    F), the second writing fp16 directly; host applies the mean's *0.25
    during the f32 upcast (exact to within fp16 rounding)
  - stores on the Sync DGE ring right after each tile's add2, fully
    overlapped with the remaining loads
  - Bacc(enable_partition_id=False, monotonic_sem_count=0) plus a
    barrier-free TileContext exit (drain + sem clears all on Sync, in
    program order) trim ~4 us of fixed init/teardown
"""

import math
import sys

for _p in ("/opt/trn_rl_repo",):
    if _p not in sys.path:
        sys.path.append(_p)

import numpy as np

# If the environment forces tracing (BASS_TRACE=1), run_bass_kernel_spmd
# imports antenv.axon_hooks, which this image lacks — stub it (only when
# absent) so the trace path degrades to "hook isn't registered" instead
# of crashing the kernel.
try:
    import antenv.axon_hooks  # noqa: F401
except ImportError:
    import types as _types

    _m = _types.ModuleType("antenv.axon_hooks")
    _m.get_axon_ntff_profile_hook = lambda: None
    _m.set_axon_ntff_profile_hook = lambda h: None
    sys.modules["antenv.axon_hooks"] = _m

import concourse.bass as bass
import concourse.mybir as mybir
from concourse import bacc
from concourse.bass_utils import run_bass_kernel_spmd
from concourse.tile import TileContext
from concourse.vector_clock import ScopedClock

N_CORES = 8
B = 4_194_304
N_PIX = 4
N_PER_CORE = B // N_CORES          # 524288 samples
P = 128                            # SBUF partitions
L = N_PER_CORE * N_PIX             # 2097152 input floats per core

# Per-tile free-dim sizes (floats per partition). DMA efficiency is
# driven by the per-partition line length (F*4 bytes): 16 KiB lines
# sustain ~400 GB/s aggregate while 4 KiB lines drop to ~300 (measured)
# — so the bulk must be 4096-wide tiles. The small FIRST tile starts
# the ACT/DVE pipeline ~8 us earlier (its act is what gates the Sin
# table load and every downstream DVE op); the decreasing TAIL keeps
# the post-last-load drain (last act + adds + store) short.
F_SCHED = [512, 4096, 4096, 4096, 2048, 1024, 512]
assert sum(F_SCHED) * P == L

# COMPUTE sub-tiling, decoupled from the DMA tiling: every sub-slice
# of a landed tile waits on the same load semaphore, so act/adds/store
# pipeline across sub-slices WITHIN a DMA tile (act of sub k+1 overlaps
# adds of sub k on DVE). This cuts the post-last-load drain from a full
# 4096-wide act+add chain to act_total + the LAST SUB's adds, without
# giving up 16 KiB DMA lines the way a finer DMA tiling would. Early
# big tiles stay unsplit: their chains hide under the load stream, and
# fewer instructions = less fixed ACT/DVE per-op overhead.
SUB_SCHED = [
    [512],
    [4096],
    [4096],
    [2048, 1024, 1024],
    [2048],
    [1024],
    [512],
]
assert all(sum(s) == F for s, F in zip(SUB_SCHED, F_SCHED))

# cos(z) = sin(pi/2 - z) for z = x*pi/255 = 2*theta: with scale=-pi/255
# and bias=+pi/2 the activation argument stays in (-pi/2, pi/2], the
# accurate domain of the HW Sin spline (it degrades badly beyond ~pi),
# and no sign fix-up is needed downstream.
_SCALE = -math.pi / 255.0
_BIAS = math.pi / 2.0


def _make_bacc() -> bacc.Bacc:
    """Construct Bacc without its init-time const-AP memsets and
    all-engine barrier. Nothing reads the four built-in const APs here
    (the activation bias is an explicitly-memset SBUF tensor, never a
    float — a float bias would route through the const APs and read
    uninitialized SBUF), and without the barrier each engine reaches its
    first kernel instruction as soon as its own runtime prolog finishes.
    The patched methods are restored before any kernel instruction is
    traced."""
    sh = bass.BassSharedVectorInterface
    saved_memset = sh.memset
    saved_barrier = bass.Bass.all_engine_barrier
    sh.memset = lambda self, ap, constant: None
    bass.Bass.all_engine_barrier = lambda self, *a, **k: None
    # (Tried: shrinking the kernel's declared semaphore pool to ~40 —
    # the NEFF exit still resets the full [2,256) space per engine, so
    # the ~6 us reset storm is hardcoded downstream. Reverted.)
    try:
        # No partition-id reads and no monotonic sems in this kernel:
        # disabling them drops the per-engine init TENSOR_LOADs (~1.2 us
        # of register setup before the first DMA dispatch).
        return bacc.Bacc(enable_partition_id=False, monotonic_sem_count=0)
    finally:
        sh.memset = saved_memset
        bass.Bass.all_engine_barrier = saved_barrier


def _fast_drain_and_barrier(self, tick_clock, wait_clock):
    """Barrier-free replacement for TileContext._drain_and_barrier.

    The stock exit emits drain + all-engine barrier + gpsimd sem clears
    + another all-engine barrier (~2-3 us of engine-skew waits at the
    very end of the kernel). The barriers only exist to order the
    gpsimd-issued clears against the other engines; issuing the drain
    AND the clears on the Sync engine instead makes program order carry
    that dependency: the drain waits on every outstanding DMA/compute
    semaphore, and the clears follow it in Sync's own stream. The
    NEFF-level postamble (which resets the whole semaphore space
    per-engine) still runs after, so cross-run state is unchanged."""
    nc = self.nc
    drain_inst = nc.sync.drain()
    wait_clock.add_sem_waits(
        drain_inst.ins, ScopedClock({None: tick_clock.global_clock})
    )
    popped = nc._tile_sem_poison_stack.pop()
    assert popped is self._sem_poison
    sems = list(self.sems.allocated().values())
    sem_nums = [s.num if hasattr(s, "num") else s for s in sems]
    for sem_range in bass.compact_to_ranges(sem_nums):
        assert nc._state.free_isdisjoint(sem_range)
        nc.sync.drain(semaphore_range=sem_range)  # dma_reset equivalent
        nc.sync.sem_clear(sem_range)
    nc._state.prepend_free_semaphores(sem_nums)
    for poison_set in nc._tile_sem_poison_stack:
        poison_set.update(sem_nums)


def _build_nc() -> bass.Bass:
    # Bacc (not raw Bass): its compile() pass generate_event_semaphores
    # splits multi-sem waits to satisfy the 1-wait-per-instruction HW limit.
    nc = _make_bacc()
    f32 = mybir.dt.float32
    f16 = mybir.dt.float16
    bf16 = mybir.dt.bfloat16
    x = nc.dram_tensor("x", [L], f32, kind="ExternalInput")
    y = nc.dram_tensor("y", [N_PER_CORE], f16, kind="ExternalOutput")

    bias_t = nc.alloc_sbuf_tensor("bias_pi2", [P, 1], f32)
    bias_ap = bias_t.ap()

    with TileContext(nc) as tc:
        # One slot per uniquely-tagged tile: no slot reuse, so no in-DMA
        # ever carries a WAR wait and the ACT sequencer can dispatch
        # every input DMA up front; slots are sized per tile (a shared
        # tag would size every slot to the largest tile).
        with tc.tile_pool(name="io", bufs=1) as pool:
            nc.gpsimd.memset(bias_ap, _BIAS)
            # All load dispatches FIRST in the ACT stream, before any
            # activation, and all on ONE queue: splitting loads across
            # two DGE queues was measured at ~300 GB/s aggregate vs
            # ~400 for a single queue (two interleaved address streams
            # lose HBM sequentiality across the shared engine pool).
            # Loads carry no sem waits, so the 7 dispatches retire
            # back-to-back before the sequencer blocks on the first
            # activation.
            # All 7 load dispatches FIRST, back-to-back: emitting any
            # compute between them (e.g. act0 after two dispatches, to
            # start DVE earlier) was measured 5+ us SLOWER — the ACT
            # sequencer blocks on act0's load wait, the load queue goes
            # briefly idle, and the stream loses its ramp/arbitration.
            in_tiles = []
            in_off = 0
            for t, F in enumerate(F_SCHED):
                x_t = x[in_off:in_off + P * F].rearrange("(p f) -> p f", p=P)
                it = pool.tile([P, F], f32, tag=f"in{t}")
                nc.scalar.dma_start(out=it[:], in_=x_t)
                in_tiles.append(it)
                in_off += P * F
            out_off = 0
            for t, F in enumerate(F_SCHED):
                it = in_tiles[t]
                C = F // N_PIX
                # Per-partition output segments are contiguous per DMA
                # TILE (stride C), so a sub-slice's outputs form a
                # COLUMN slice of the tile's (P, C) view — not a packed
                # contiguous range.
                y_tile = y[out_off:out_off + P * C].rearrange(
                    "(p c) -> p c", p=P
                )
                # NOTE: routing the Sin output through a separate bf16
                # tile (for 2x 16-bit DVE adds) was measured to produce
                # INTERMITTENT wrong results (rel err 0.1 on 1 of 3
                # runs — a missed dependency race) with no speed gain;
                # keep the in-place f32 activation.
                st = pool.tile([P, C], f16, tag=f"sum{t}")
                g = 0
                for s, S in enumerate(SUB_SCHED[t]):
                    SC = S // N_PIX
                    sub = it[:, g:g + S]
                    nc.scalar.activation(
                        sub, sub, mybir.ActivationFunctionType.Sin,
                        bias=bias_ap, scale=_SCALE,
                    )
                    # Grouped sum of 4 as two pairwise adds; the second
                    # writes fp16 directly (DVE converts on write) into
                    # this sub's column slice of the tile's output
                    # buffer; the host applies the mean's *0.25 during
                    # the f32 upcast. Unique tags: no WAR waits.
                    pt = pool.tile([P, S // 2], f32, tag=f"pair{t}_{s}")
                    # NOTE: offloading add1 of the early 4096s to the
                    # idle GpSimd engine (to cut DVE busy) measured
                    # neutral — the tail is act-queue bound, not DVE-
                    # capacity bound — and one multi-execution session
                    # with it hit NRT_EXEC_UNIT_UNRECOVERABLE; keep the
                    # adds on DVE.
                    nc.vector.tensor_add(
                        pt[:], it[:, g:g + S:2], it[:, g + 1:g + S:2]
                    )
                    nc.vector.tensor_add(
                        st[:, g // N_PIX:g // N_PIX + SC],
                        pt[:, 0:S // 2:2], pt[:, 1:S // 2:2]
                    )
                    g += S
                # ONE store per DMA tile (not per sub), after the last
                # sub's add2: fewer serialized dispatches on the Sync
                # ring; the subs' adds complete nearly together anyway.
                nc.sync.dma_start(out=y_tile, in_=st[:])
                out_off += P * C
    nc.finalize()
    return nc


def _build_nc_patched() -> bass.Bass:
    saved = TileContext._drain_and_barrier
    TileContext._drain_and_barrier = _fast_drain_and_barrier
    try:
        return _build_nc()
    finally:
        TileContext._drain_and_barrier = saved


_NC_CACHE = None


def _get_nc() -> bass.Bass:
    global _NC_CACHE
    if _NC_CACHE is None:
        _NC_CACHE = _build_nc_patched()
    return _NC_CACHE


def _run(x: np.ndarray, **spmd_kwargs):
    """x: (B, 4) float32. Returns (full_output, BassKernelResults)."""
    shards = x.reshape(N_CORES, L)
    in_maps = [{"x": shards[i]} for i in range(N_CORES)]
    res = run_bass_kernel_spmd(_get_nc(), in_maps, list(range(N_CORES)), **spmd_kwargs)
    out = np.zeros((B, 3), dtype=np.float32)
    for i, r in enumerate(res.results):
        # Device ships the per-sample sum of the 4 cosines (fp16); the
        # mean's *0.25 is applied here during the f32 upcast.
        out[i * N_PER_CORE:(i + 1) * N_PER_CORE, 2] = np.asarray(
            r["y"], dtype=np.float32
        ).reshape(N_PER_CORE) * np.float32(0.25)
    return out, res


def kernel(**inputs: np.ndarray) -> np.ndarray:
    x = np.ascontiguousarray(
        np.asarray(inputs["inputs"], dtype=np.float32)
    ).reshape(B, N_PIX)
    out, _ = _run(x)
    return out
